# revision 2
# baseline (speedup 1.0000x reference)
"""GraphSAGE-style 2-layer minibatch forward on 8 trn2 NeuronCores.

Strategy: data-parallel over the 4096 target nodes (512 per core); the
1M x 128 feature table and the small weight matrices are replicated to
every core, so there is no cross-core communication.  Each core runs the
same single-device program; all 8 are dispatched asynchronously.

The mean-over-neighbors is folded into pre-scaled weight halves so the
device only needs sums:
    relu(W0 @ [x_self ; mean_j x_j]) == relu(x_self @ W0s.T + (sum_j x_j) @ (W0n/25).T)
"""
import numpy as np
import jax
import jax.numpy as jnp

N = 1_000_000
F = 128
H = 128
B = 4096
S1 = 10
S0 = 25
NC = 8
BC = B // NC

_EPS = 1e-12


def _l2norm(h):
    n = jnp.linalg.norm(h, axis=-1, keepdims=True)
    return h / jnp.maximum(n, _EPS)


def _forward(features, w0s, w0n, b0, w1s, w1n, b1, lvl1, nbr0):
    # lvl1: [BC, 1+S1] int32, nbr0: [BC, (1+S1)*S0] int32
    h0_self = features[lvl1]                                    # [BC, 11, F]
    h0_nbr = features[nbr0].reshape(BC, 1 + S1, S0, F).sum(2)   # [BC, 11, F]
    pre1 = h0_self @ w0s + h0_nbr @ w0n + b0
    h1 = _l2norm(jax.nn.relu(pre1))                             # [BC, 11, H]
    pre2 = h1[:, 0, :] @ w1s + h1[:, 1:, :].sum(1) @ w1n + b1
    return _l2norm(jax.nn.relu(pre2))                           # [BC, H]


_jit_fns = {}


def _fn_for(dev):
    if dev not in _jit_fns:
        _jit_fns[dev] = jax.jit(_forward)
    return _jit_fns[dev]


def _prep(features, W0, b0, W1, b1, nodes, nbr1, nbr0):
    features = np.asarray(features, dtype=np.float32)
    W0 = np.asarray(W0, dtype=np.float32)
    W1 = np.asarray(W1, dtype=np.float32)
    b0 = np.asarray(b0, dtype=np.float32)
    b1 = np.asarray(b1, dtype=np.float32)
    lvl1 = np.concatenate([np.asarray(nodes)[:, None], np.asarray(nbr1)], axis=1).astype(np.int32)
    nbr0f = np.asarray(nbr0).reshape(B, (1 + S1) * S0).astype(np.int32)
    w0s = np.ascontiguousarray(W0[:, :F].T)
    w0n = np.ascontiguousarray(W0[:, F:].T) / S0
    w1s = np.ascontiguousarray(W1[:, :H].T)
    w1n = np.ascontiguousarray(W1[:, H:].T) / S1
    return features, w0s, w0n, b0, w1s, w1n, b1, lvl1, nbr0f


def _place_args(prep):
    """Ship per-core argument lists to devices; returns list of arg tuples."""
    features, w0s, w0n, b0, w1s, w1n, b1, lvl1, nbr0f = prep
    devs = jax.devices()[:NC]
    per_core = []
    for c, d in enumerate(devs):
        sl = slice(c * BC, (c + 1) * BC)
        args = (
            jax.device_put(features, d),
            jax.device_put(w0s, d), jax.device_put(w0n, d), jax.device_put(b0, d),
            jax.device_put(w1s, d), jax.device_put(w1n, d), jax.device_put(b1, d),
            jax.device_put(lvl1[sl], d), jax.device_put(nbr0f[sl], d),
        )
        per_core.append(args)
    return per_core


def _run(per_core):
    devs = jax.devices()[:NC]
    outs = [_fn_for(d)(*args) for d, args in zip(devs, per_core)]
    return np.concatenate([np.asarray(jax.block_until_ready(o)) for o in outs], axis=0)


def kernel(features, W0, b0, W1, b1, nodes, nbr1, nbr0):
    prep = _prep(features, W0, b0, W1, b1, nodes, nbr1, nbr0)
    per_core = _place_args(prep)
    return _run(per_core)
